# revision 1
# baseline (speedup 1.0000x reference)
"""GNN attention (GAT-style single-target-node) kernel for 8 Trainium2 cores.

Problem:  x [32, 50000, 64], a [128, 1], node_index scalar, adj_mask [50000]
  tgt_score = x[:, idx] @ a[:64]                             # [B]
  e = leaky_relu(tgt_score[:, None] + x @ a[64:], 0.01)      # [B, N]
  attention = softmax(where(adj>0, e, -9e15), axis=1) * adj  # [B, N]

Sharding: data-parallel over batch (32 = 8 cores x 4 batches/core). Each
core computes complete softmax rows, so no cross-core reductions.

Per-core layout: nodes tile as [128 partitions, TK nodes, 64 feats];
scores accumulate into a [128, 400] grid (TF full tiles of TK plus a
[53, 16] tail).  Dot products: elementwise multiply against a replicated
a_src then a grouped free-axis reduce.  The multiply is split between
GPSIMD (k < GK) and DVE (k >= GK) to balance engine load, since DVE also
owns the reduces.  Softmax cross-partition max/sum go through a PE
transpose + ones-matmul broadcast (PE is otherwise idle).
"""

import numpy as np
from contextlib import ExitStack

import jax
from jax.sharding import Mesh, PartitionSpec
from jax.experimental.shard_map import shard_map

import concourse.bass as bass
import concourse.bacc as bacc
import concourse.tile as tile
from concourse.tile import add_dep_helper
from concourse import mybir
from concourse.bass2jax import _bass_exec_p, install_neuronx_cc_hook

B, N, D = 32, 50000, 64
NCORES = 8
BPC = B // NCORES            # batches per core
TK = 64                      # nodes per partition per full tile (divides 384)
TF = 384 // TK               # full tiles, TF * 128 * TK = 49152 nodes
NFULL = TF * 128 * TK        # 49152
KT = 16                      # tail: nodes per partition
PT = (N - NFULL) // KT       # 53 partitions in tail tile
CF = TF * TK                 # 384 full-score columns
COLS = CF + KT               # 400 score columns
GK = 48                      # k < GK multiplies on GPSIMD, rest on DVE
GKT = 12                     # same split for the tail tile
GA = 0                       # k-cols of tree level-1 on GPSIMD (0: the cross-
                             # engine dep inside the tree costs more than it
                             # saves -- GPSIMD's 4-deep FIFO delays level 2)
REDUCE_MODE = "tree"         # "grouped": one reduce_sum; "tree": log2(D) adds
XB = 4                       # x-tile pool depth
PB = 3                       # product-tile pool depth
NEG = -9.0e15

F32 = mybir.dt.float32
AX = mybir.AxisListType
OP = mybir.AluOpType
ACT = mybir.ActivationFunctionType

TRACE = False                # set True (e.g. from test.py) to neuron-profile
LAST_RUN = None              # BassKernelResults of the most recent run

_CACHE = {}


def _build(reps=1):
    nc = bacc.Bacc(trn_type="TRN2", enable_partition_id=False,
                   num_devices=NCORES)
    xs = nc.dram_tensor("xs", [BPC, N, D], F32, kind="ExternalInput").ap()
    tgt_d = nc.dram_tensor("tgtvec", [128, BPC], F32, kind="ExternalInput").ap()
    arep_d = nc.dram_tensor("arep", [128, TK * D], F32, kind="ExternalInput").ap()
    mb_d = nc.dram_tensor("mbgrid", [128, COLS], F32, kind="ExternalInput").ap()
    id_d = nc.dram_tensor("ident", [128, 128], F32, kind="ExternalInput").ap()
    on_d = nc.dram_tensor("onesr", [1, 128], F32, kind="ExternalInput").ap()
    attn = nc.dram_tensor("attn", [BPC, N], F32, kind="ExternalOutput").ap()

    with tile.TileContext(nc) as tc, ExitStack() as ctx:
        singles = ctx.enter_context(tc.tile_pool(name="singles", bufs=1))
        xpool = ctx.enter_context(tc.tile_pool(name="xpool", bufs=XB))
        ppool = ctx.enter_context(tc.tile_pool(name="ppool", bufs=PB))
        spool = ctx.enter_context(tc.tile_pool(name="spool", bufs=2))
        epool = ctx.enter_context(tc.tile_pool(name="epool", bufs=2))
        stat = ctx.enter_context(tc.tile_pool(name="stat", bufs=8))
        pspool = ctx.enter_context(tc.tile_pool(name="ps", bufs=4, space="PSUM"))

        arep_sb = singles.tile([128, TK * D], F32)
        # the pipeline-fill quarters only read the first TK/4 k-columns of the
        # replicated a_src, so land those first and the bulk later
        qcols = (TK // 4) * D
        with tc.high_priority():
            nc.sync.dma_start(out=arep_sb[:, :qcols], in_=arep_d[:, :qcols])
        nc.sync.dma_start(out=arep_sb[:, qcols:], in_=arep_d[:, qcols:])
        arep3 = arep_sb[:].rearrange("p (k d) -> p k d", d=D)
        mb_sb = singles.tile([128, COLS], F32)
        nc.sync.dma_start(out=mb_sb, in_=mb_d)
        tgt_sb = singles.tile([128, BPC], F32)
        nc.sync.dma_start(out=tgt_sb, in_=tgt_d)
        ident = singles.tile([128, 128], F32)
        nc.sync.dma_start(out=ident, in_=id_d)
        onesr = singles.tile([1, 128], F32)
        nc.sync.dma_start(out=onesr, in_=on_d)

        def cross_partition(vec, op):
            """[128,1] per-partition stats -> [1,1] global (PE transpose)."""
            tp = pspool.tile([1, 128], F32, tag="ps")
            nc.tensor.transpose(tp, vec, ident)
            ct = stat.tile([1, 128], F32, tag="ct")
            nc.vector.tensor_copy(ct, tp)
            g1 = stat.tile([1, 1], F32, tag="g1")
            nc.vector.tensor_reduce(g1, ct, axis=AX.X, op=op)
            return g1

        def bcast_partitions(s1, tag):
            """[1,1] scalar -> [128,1] replicated (ones-matmul)."""
            bp = pspool.tile([128, 1], F32, tag="ps")
            nc.tensor.matmul(bp, onesr, s1, start=True, stop=True)
            out = stat.tile([128, 1], F32, tag=tag)
            nc.vector.tensor_copy(out, bp)
            return out

        def mul_split(pr, xt, ar, gk, k):
            if gk > 0:
                nc.gpsimd.tensor_mul(pr[:, :gk, :], xt[:, :gk, :], ar[:, :gk, :])
            if gk < k:
                nc.vector.tensor_mul(pr[:, gk:, :], xt[:, gk:, :], ar[:, gk:, :])

        def dot_reduce(sbcols, pr, ga=0):
            """Sum pr [128, k, 64] over the last axis into sbcols [128, k].
            ga > 0 routes the first ga k-columns of the top tree level to
            GPSIMD to shave the DVE's busy time."""
            if REDUCE_MODE == "grouped":
                nc.vector.reduce_sum(sbcols, pr, axis=AX.X)
                return
            w = D // 2
            if ga > 0:
                nc.gpsimd.tensor_add(pr[:, :ga, 0:w], pr[:, :ga, 0:w],
                                     pr[:, :ga, w:2 * w])
                nc.vector.tensor_add(pr[:, ga:, 0:w], pr[:, ga:, 0:w],
                                     pr[:, ga:, w:2 * w])
                w //= 2
            while w > 1:
                nc.vector.tensor_add(pr[:, :, 0:w], pr[:, :, 0:w],
                                     pr[:, :, w:2 * w])
                w //= 2
            nc.vector.tensor_add(sbcols, pr[:, :, 0], pr[:, :, 1])

        first_tile = True
        quarter_dmas = []
        full_dmas_ordered = 0
        for _ in range(reps):
            for b in range(BPC):
                sb = spool.tile([128, COLS], F32)
                # tail-tile slots with no node behind them: keep them finite so
                # the masked add (-9e15) sends them to zero probability.  (The
                # tail reduce overwrites partitions < PT afterwards.)
                nc.vector.memset(sb[:, CF:], 0.0)
                for t in range(TF):
                    xt = xpool.tile([128, TK, D], F32)
                    src = xs[b, t * 128 * TK:(t + 1) * 128 * TK, :] \
                        .rearrange("(p k) d -> p k d", p=128)
                    if first_tile:
                        # split the very first tile into quarters (own DMAs,
                        # all-DVE multiply) so compute starts ~4x earlier
                        # instead of stalling through one 2 MB DMA + GPSIMD;
                        # high_priority keeps the quarters ahead of the next
                        # tiles' full-size DMAs in the DMA queue
                        first_tile = False
                        q = TK // 4
                        with tc.high_priority():
                            for i in range(4):
                                qd = nc.sync.dma_start(
                                    out=xt[:, i * q:(i + 1) * q, :],
                                    in_=src[:, i * q:(i + 1) * q, :])
                                quarter_dmas.append(qd.ins)
                                pr = ppool.tile([128, q, D], F32, tag="prq")
                                nc.vector.tensor_mul(
                                    pr, xt[:, i * q:(i + 1) * q, :],
                                    arep3[:, :q, :])
                                dot_reduce(
                                    sb[:, t * TK + i * q:t * TK + (i + 1) * q],
                                    pr)
                        continue
                    fd = nc.sync.dma_start(out=xt, in_=src)
                    if quarter_dmas and full_dmas_ordered < 3:
                        # keep the fill quarters ahead of the first full-size
                        # DMAs in the queue (scheduler otherwise reorders)
                        add_dep_helper(fd.ins, quarter_dmas[-1], sync=False,
                                       reason="pipeline fill: quarters first")
                        full_dmas_ordered += 1
                    pr = ppool.tile([128, TK, D], F32)
                    mul_split(pr, xt, arep3, GK, TK)
                    dot_reduce(sb[:, t * TK:(t + 1) * TK], pr, ga=GA)
                # tail tile: 848 nodes = [53 partitions, 16 nodes, 64 feats]
                xt_t = xpool.tile([128, KT, D], F32)
                nc.sync.dma_start(
                    out=xt_t[:PT],
                    in_=xs[b, NFULL:N, :].rearrange("(p k) d -> p k d", p=PT),
                )
                pr_t = ppool.tile([128, KT, D], F32)
                mul_split(pr_t[:PT], xt_t[:PT], arep3[:PT, :KT, :], GKT, KT)
                dot_reduce(sb[:PT, CF:], pr_t[:PT])

                # z = leaky_relu(scores + tgt, 0.01) + mask_bias.  Scores are
                # O(10), so exp(z) cannot overflow fp32 and the usual
                # max-subtraction is unnecessary (softmax is shift-invariant);
                # skipping it removes a whole PE/DVE round-trip per batch.
                z = epool.tile([128, COLS], F32)
                nc.vector.tensor_scalar_add(z, sb, tgt_sb[:, b:b + 1])
                nc.vector.scalar_tensor_tensor(z, z, 0.01, z,
                                               op0=OP.mult, op1=OP.max)
                nc.vector.tensor_add(z, z, mb_sb)

                pb = epool.tile([128, COLS], F32)
                srow = stat.tile([128, 1], F32)
                nc.scalar.activation(pb, z, ACT.Exp, bias=0.0, scale=1.0,
                                     accum_out=srow)
                gsum1 = cross_partition(srow, OP.add)
                rec1 = stat.tile([1, 1], F32)
                nc.vector.reciprocal(rec1, gsum1)
                rec = bcast_partitions(rec1, "rec")
                nc.vector.tensor_scalar_mul(pb, pb, rec)

                nc.sync.dma_start(
                    out=attn[b, 0:NFULL].rearrange("(t p k) -> p t k",
                                                   p=128, k=TK),
                    in_=pb[:, 0:CF].rearrange("p (t k) -> p t k", t=TF),
                )
                nc.sync.dma_start(
                    out=attn[b, NFULL:N].rearrange("(p k) -> p k", k=KT),
                    in_=pb[:PT, CF:],
                )
    nc.compile()
    return nc


def _host_prep(x, a, node_index, adj_mask):
    x = np.asarray(x, dtype=np.float32)
    a = np.asarray(a, dtype=np.float32).reshape(2 * D)
    adj = np.asarray(adj_mask).astype(np.int64)
    idx = int(node_index)
    a_tgt, a_src = a[:D], a[D:]

    tgt = (x[:, idx, :] @ a_tgt).astype(np.float32)          # [B]
    arep = np.tile(a_src, (128, TK)).astype(np.float32)      # [128, TK*D]

    mb = np.full((128, COLS), NEG, np.float32)
    m_full = adj[:NFULL].reshape(TF, 128, TK)
    mb[:, :CF] = np.where(
        m_full.transpose(1, 0, 2).reshape(128, CF) > 0, 0.0, NEG)
    m_tail = adj[NFULL:].reshape(PT, KT)
    mb[:PT, CF:] = np.where(m_tail > 0, 0.0, NEG)
    ident = np.eye(128, dtype=np.float32)
    onesr = np.ones((1, 128), dtype=np.float32)
    return x, tgt, arep, mb, ident, onesr


def _in_maps(x, tgt, arep, mb, ident, onesr):
    maps = []
    for c in range(NCORES):
        tv = np.tile(tgt[c * BPC:(c + 1) * BPC][None, :],
                     (128, 1)).astype(np.float32)
        maps.append({
            "xs": np.ascontiguousarray(x[c * BPC:(c + 1) * BPC]),
            "tgtvec": tv,
            "arep": arep,
            "mbgrid": mb,
            "ident": ident,
            "onesr": onesr,
        })
    return maps


def _runner():
    """Build the Bass program once and wrap its NEFF custom call in a jitted
    shard_map over the 8 cores.  Cached so repeat kernel() calls only pay
    input upload + execution, not re-tracing/compiling."""
    if "runner" in _CACHE:
        return _CACHE["runner"]
    install_neuronx_cc_hook()
    nc = _CACHE.setdefault("nc", _build())
    in_names, out_names, out_avals, zero_shapes = [], [], [], []
    for alloc in nc.m.functions[0].allocations:
        if not isinstance(alloc, mybir.MemoryLocationSet):
            continue
        name = alloc.memorylocations[0].name
        if alloc.kind == "ExternalInput":
            in_names.append(name)
        elif alloc.kind == "ExternalOutput":
            out_names.append(name)
            shape = tuple(alloc.tensor_shape)
            dtype = mybir.dt.np(alloc.dtype)
            out_avals.append(jax.core.ShapedArray(shape, dtype))
            zero_shapes.append((shape, dtype))

    def _body(*args):
        return tuple(_bass_exec_p.bind(
            *args,
            out_avals=tuple(out_avals),
            in_names=tuple(in_names + out_names),
            out_names=tuple(out_names),
            lowering_input_output_aliases=(),
            sim_require_finite=True,
            sim_require_nnan=True,
            nc=nc,
        ))

    mesh = Mesh(np.asarray(jax.devices()[:NCORES]), ("core",))
    nin = len(in_names) + len(out_names)
    sharded = jax.jit(shard_map(
        _body, mesh=mesh,
        in_specs=(PartitionSpec("core"),) * nin,
        out_specs=(PartitionSpec("core"),) * len(out_names),
        check_rep=False))
    _CACHE["runner"] = (sharded, in_names, out_names, zero_shapes)
    return _CACHE["runner"]


def kernel(x, a, node_index, adj_mask):
    global LAST_RUN
    prep = _host_prep(x, a, node_index, adj_mask)
    maps = _in_maps(*prep)
    sharded, in_names, out_names, zero_shapes = _runner()
    # concat of the 8 per-core xs shards is exactly the full x — skip the copy
    ins = [prep[0] if nm == "xs" else
           np.concatenate([m[nm] for m in maps], axis=0) for nm in in_names]
    zeros = [np.zeros((NCORES * s[0], *s[1:]), d) for s, d in zero_shapes]
    outs = sharded(*ins, *zeros)
    LAST_RUN = outs
    attn = np.asarray(outs[out_names.index("attn")])  # [NCORES*BPC, N]
    return attn.reshape(B, N)



# revision 4
# speedup vs baseline: 2.6464x; 2.6464x over previous
"""GNN attention (GAT-style single-target-node) kernel for 8 Trainium2 cores.

Problem:  x [32, 50000, 64], a [128, 1], node_index scalar, adj_mask [50000]
  tgt_score = x[:, idx] @ a[:64]                             # [B]
  e = leaky_relu(tgt_score[:, None] + x @ a[64:], 0.01)      # [B, N]
  attention = softmax(where(adj>0, e, -9e15), axis=1) * adj  # [B, N]

Sharding: data-parallel over batch (32 = 8 cores x 4 batches/core).

The x @ a_src dot product runs on the TensorEngine: the host packs x
(cast to bf16, zero-padded to 50176 nodes) into per-batch SBUF tile
images laid out [128 partitions = (node-parity, feature), columns] so
that 64 accumulating matmuls against a block-sparse replicated-a weight
matrix land the scores directly as a [128, 392] PSUM grid (psum[p, t] =
score of node p*392 + t).  Each matmul's stationary operand has a_src
in two one-hot columns, so matmul m adds chunk-2m/2m+1 scores into PSUM
partitions 2m/2m+1 and zeros elsewhere.  ScalarE applies the target
bias + exp (with per-partition row sums), DVE does leaky-relu/mask/
final scale, and PE matmuls handle the cross-partition softmax sum and
broadcast.  bf16 halves HBM traffic; fp32 would be ~2e-6 rel err, bf16
is ~4e-3, both well under the 2e-2 gate.
"""

import numpy as np
from contextlib import ExitStack

import ml_dtypes
import jax
from jax.sharding import Mesh, PartitionSpec
from jax.experimental.shard_map import shard_map

import concourse.bass as bass
import concourse.bacc as bacc
import concourse.tile as tile
from concourse import mybir
from concourse.bass2jax import _bass_exec_p, install_neuronx_cc_hook

B, N, D = 32, 50000, 64
NCORES = 8
BPC = B // NCORES            # batches per core
C = 392                      # score-grid columns (nodes per partition chunk)
G = 128                      # node chunks per batch = psum partitions
NPAD = C * G                 # 50176 padded nodes
NM = 64                      # matmuls per batch, 2 chunks (C columns) each
NJ = 8                       # input DMA blocks per batch
MPJ = NM // NJ               # matmuls covered per DMA block
CB = MPJ * C                 # SBUF columns per DMA block
NEG = -9.0e15

F32 = mybir.dt.float32
BF16 = mybir.dt.bfloat16
NPBF16 = ml_dtypes.bfloat16
OP = mybir.AluOpType
ACT = mybir.ActivationFunctionType

TRACE = False
LAST_RUN = None

_CACHE = {}


def _build(reps=1):
    nc = bacc.Bacc(trn_type="TRN2", enable_partition_id=False,
                   num_devices=NCORES)
    xp_d = nc.dram_tensor("xp", [BPC, NJ, 128, CB], BF16,
                          kind="ExternalInput").ap()
    w_d = nc.dram_tensor("wts", [128, NM * 128], BF16,
                         kind="ExternalInput").ap()
    tgt_d = nc.dram_tensor("tgtvec", [128, BPC], F32, kind="ExternalInput").ap()
    mb_d = nc.dram_tensor("mbgrid", [128, C], F32, kind="ExternalInput").ap()
    on_d = nc.dram_tensor("onesr", [1, 128], F32, kind="ExternalInput").ap()
    oc_d = nc.dram_tensor("onescol", [128, 1], F32, kind="ExternalInput").ap()
    attn = nc.dram_tensor("attn", [BPC, N], F32, kind="ExternalOutput").ap()

    with tile.TileContext(nc) as tc, ExitStack() as ctx:
        singles = ctx.enter_context(tc.tile_pool(name="singles", bufs=1))
        xpool = ctx.enter_context(tc.tile_pool(name="xpool", bufs=2))
        zpool = ctx.enter_context(tc.tile_pool(name="zpool", bufs=2))
        ppool = ctx.enter_context(tc.tile_pool(name="ppool", bufs=2))
        stat = ctx.enter_context(tc.tile_pool(name="stat", bufs=8))
        psgrid = ctx.enter_context(tc.tile_pool(name="psg", bufs=3,
                                                space="PSUM"))
        pssml = ctx.enter_context(tc.tile_pool(name="pss", bufs=2,
                                               space="PSUM"))

        w_sb = singles.tile([128, NM * 128], BF16)
        # chunked so the first matmuls don't wait on the full 2 MB load
        wq = NM * 128 // 4
        for q in range(4):
            nc.sync.dma_start(out=w_sb[:, q * wq:(q + 1) * wq],
                              in_=w_d[:, q * wq:(q + 1) * wq])
        mb_sb = singles.tile([128, C], F32)
        nc.sync.dma_start(out=mb_sb, in_=mb_d)
        tgt_sb = singles.tile([128, BPC], F32)
        nc.sync.dma_start(out=tgt_sb, in_=tgt_d)
        onesr = singles.tile([1, 128], F32)
        nc.sync.dma_start(out=onesr, in_=on_d)
        onesc = singles.tile([128, 1], F32)
        nc.sync.dma_start(out=onesc, in_=oc_d)

        for _ in range(reps):
            for b in range(BPC):
                xb = xpool.tile([128, NJ * CB], BF16)
                for j in range(NJ):
                    nc.sync.dma_start(out=xb[:, j * CB:(j + 1) * CB],
                                      in_=xp_d[b, j])
                ps = psgrid.tile([128, C], F32)
                for m in range(NM):
                    nc.tensor.matmul(ps,
                                     w_sb[:, m * 128:(m + 1) * 128],
                                     xb[:, m * C:(m + 1) * C],
                                     start=(m == 0), stop=(m == NM - 1))
                # z = scores + tgt (ScalarE copy w/ per-partition bias)
                z = zpool.tile([128, C], F32)
                nc.scalar.activation(z, ps, ACT.Identity,
                                     bias=tgt_sb[:, b:b + 1], scale=1.0)
                # leaky_relu + additive mask (-9e15 at masked/pad slots)
                nc.vector.scalar_tensor_tensor(z, z, 0.01, z,
                                               op0=OP.mult, op1=OP.max)
                nc.vector.tensor_add(z, z, mb_sb)
                # exp + per-partition row sums; scores are O(10) so no
                # max-subtraction is needed (softmax is shift-invariant)
                pb = ppool.tile([128, C], F32)
                srow = stat.tile([128, 1], F32)
                nc.scalar.activation(pb, z, ACT.Exp, bias=0.0, scale=1.0,
                                     accum_out=srow)
                # global sum via ones-matmul, reciprocal, broadcast back
                gs = pssml.tile([1, 1], F32, tag="gs")
                nc.tensor.matmul(gs, srow, onesc, start=True, stop=True)
                rec1 = stat.tile([1, 1], F32, tag="rec1")
                nc.vector.reciprocal(rec1, gs)
                bc = pssml.tile([128, 1], F32, tag="bc")
                nc.tensor.matmul(bc, onesr, rec1, start=True, stop=True)
                rec = stat.tile([128, 1], F32, tag="rec")
                nc.vector.tensor_copy(rec, bc)
                nc.vector.tensor_scalar_mul(pb, pb, rec)

                nc.sync.dma_start(
                    out=attn[b, 0:(G - 1) * C].rearrange("(p t) -> p t", t=C),
                    in_=pb[0:G - 1, :])
                nc.sync.dma_start(
                    out=attn[b, (G - 1) * C:N].rearrange("(p t) -> p t", p=1),
                    in_=pb[G - 1:G, 0:N - (G - 1) * C])
    nc.compile()
    return nc


def _host_prep(x, a, node_index, adj_mask):
    x = np.asarray(x, dtype=np.float32)
    a = np.asarray(a, dtype=np.float32).reshape(2 * D)
    adj = np.asarray(adj_mask).astype(np.int64)
    idx = int(node_index)
    a_tgt, a_src = a[:D], a[D:]

    tgt = (x[:, idx, :] @ a_tgt).astype(np.float32)          # [B]
    ab = a_src.astype(NPBF16)

    # x -> bf16, pad to NPAD nodes, pack into per-batch SBUF tile images:
    # xp[b, j, n2*64+d, mw*C+t] = x[b, (2*(MPJ*j+mw)+n2)*C + t, d]
    xq = np.zeros((B, NPAD, D), dtype=NPBF16)
    xq[:, :N] = x
    xp = np.ascontiguousarray(
        xq.reshape(B, NJ, MPJ, 2, C, D).transpose(0, 1, 3, 5, 2, 4)
    ).reshape(B, NJ, 128, CB)

    # block-sparse weights: W[n2*64+d, m*128+p] = a_src[d] iff p == 2m+n2
    W3 = np.zeros((2, D, NM, 128), dtype=NPBF16)
    mi = np.arange(NM)
    W3[0, :, mi, 2 * mi] = ab[None, :]
    W3[1, :, mi, 2 * mi + 1] = ab[None, :]
    W = W3.reshape(128, NM * 128)

    # additive mask grid in the score layout (node n = p*C + t)
    adjp = np.zeros(NPAD, dtype=np.int64)
    adjp[:N] = adj
    mb = np.where(adjp.reshape(G, C) > 0, 0.0, NEG).astype(np.float32)

    onesr = np.ones((1, 128), dtype=np.float32)
    onesc = np.ones((128, 1), dtype=np.float32)
    return xp, tgt, W, mb, onesr, onesc


def _in_maps(xp, tgt, W, mb, onesr, onesc):
    maps = []
    for c in range(NCORES):
        tv = np.tile(tgt[c * BPC:(c + 1) * BPC][None, :],
                     (128, 1)).astype(np.float32)
        maps.append({
            "xp": xp[c * BPC:(c + 1) * BPC],
            "wts": W,
            "tgtvec": tv,
            "mbgrid": mb,
            "onesr": onesr,
            "onescol": onesc,
        })
    return maps


def _runner():
    """Build the Bass program once and wrap its NEFF custom call in a jitted
    shard_map over the 8 cores."""
    if "runner" in _CACHE:
        return _CACHE["runner"]
    install_neuronx_cc_hook()
    nc = _CACHE.setdefault("nc", _build())
    in_names, out_names, out_avals, zero_shapes = [], [], [], []
    for alloc in nc.m.functions[0].allocations:
        if not isinstance(alloc, mybir.MemoryLocationSet):
            continue
        name = alloc.memorylocations[0].name
        if alloc.kind == "ExternalInput":
            in_names.append(name)
        elif alloc.kind == "ExternalOutput":
            out_names.append(name)
            shape = tuple(alloc.tensor_shape)
            dtype = mybir.dt.np(alloc.dtype)
            out_avals.append(jax.core.ShapedArray(shape, dtype))
            zero_shapes.append((shape, dtype))

    def _body(*args):
        return tuple(_bass_exec_p.bind(
            *args,
            out_avals=tuple(out_avals),
            in_names=tuple(in_names + out_names),
            out_names=tuple(out_names),
            lowering_input_output_aliases=(),
            sim_require_finite=True,
            sim_require_nnan=True,
            nc=nc,
        ))

    mesh = Mesh(np.asarray(jax.devices()[:NCORES]), ("core",))
    nin = len(in_names) + len(out_names)
    sharded = jax.jit(shard_map(
        _body, mesh=mesh,
        in_specs=(PartitionSpec("core"),) * nin,
        out_specs=(PartitionSpec("core"),) * len(out_names),
        check_rep=False))
    _CACHE["runner"] = (sharded, in_names, out_names, zero_shapes)
    return _CACHE["runner"]


def kernel(x, a, node_index, adj_mask):
    global LAST_RUN
    prep = _host_prep(x, a, node_index, adj_mask)
    maps = _in_maps(*prep)
    sharded, in_names, out_names, zero_shapes = _runner()
    # concat of the 8 per-core xp shards is exactly the full packed x
    ins = [prep[0] if nm == "xp" else
           np.concatenate([m[nm] for m in maps], axis=0) for nm in in_names]
    zeros = [np.zeros((NCORES * s[0], *s[1:]), d) for s, d in zero_shapes]
    outs = sharded(*ins, *zeros)
    LAST_RUN = outs
    attn = np.asarray(outs[out_names.index("attn")])  # [NCORES*BPC, N]
    return attn.reshape(B, N)


# revision 5
# speedup vs baseline: 6.1354x; 2.3184x over previous
"""GNN attention (GAT-style single-target-node) kernel for 8 Trainium2 cores.

Problem:  x [32, 50000, 64], a [128, 1], node_index scalar, adj_mask [50000]
  tgt_score = x[:, idx] @ a[:64]                             # [B]
  e = leaky_relu(tgt_score[:, None] + x @ a[64:], 0.01)      # [B, N]
  attention = softmax(where(adj>0, e, -9e15), axis=1) * adj  # [B, N]

Sharding: data-parallel over batch (32 = 8 cores x 4 batches/core).

Masked nodes (adj_mask == 0) contribute exactly nothing: their attention
is 0 and they drop out of the softmax sum (exp(-9e15) == 0 in fp32).  The
host therefore compacts x to the kept nodes only (halving HBM traffic for
a ~50% mask), and scatters the dense device output back into the full
[B, N] grid.  Geometry is specialized to the runtime kept-count and the
compiled program is cached per size.

The x @ a_src dot product runs on the TensorEngine: the host packs the
kept nodes (cast to bf16, zero-padded to 128*C) into per-batch SBUF tile
images laid out [128 partitions = (node-parity, feature), columns] so
that 64 accumulating matmuls against a block-sparse replicated-a weight
matrix land the scores directly as a [128, C] PSUM grid (psum[p, t] =
score of kept node p*C + t).  Each matmul's stationary operand has a_src
in two one-hot columns, so matmul m adds chunk-2m/2m+1 scores into PSUM
partitions 2m/2m+1 and zeros elsewhere.  ScalarE applies the target
bias + exp (with per-partition row sums), DVE does leaky-relu/pad-mask/
final scale, and PE matmuls handle the cross-partition softmax sum and
broadcast.  Input DMAs alternate between the SP and ACT HWDGE rings.
bf16 x/a gives ~4e-3 rel err vs the 2e-2 gate (fp32 accumulation).
"""

import math
import numpy as np
from contextlib import ExitStack

import ml_dtypes
import jax
from jax.sharding import Mesh, PartitionSpec
from jax.experimental.shard_map import shard_map

import concourse.bass as bass
import concourse.bacc as bacc
import concourse.tile as tile
from concourse import mybir
from concourse.bass2jax import _bass_exec_p, install_neuronx_cc_hook

B, N, D = 32, 50000, 64
NCORES = 8
BPC = B // NCORES            # batches per core
G = 128                      # node chunks per batch = psum partitions
NM = 64                      # matmuls per batch, 2 chunks each
NJ = 4                       # input DMA blocks per batch
MPJ = NM // NJ               # matmuls covered per DMA block
NEG = -9.0e15

F32 = mybir.dt.float32
BF16 = mybir.dt.bfloat16
NPBF16 = ml_dtypes.bfloat16
OP = mybir.AluOpType
ACT = mybir.ActivationFunctionType

TRACE = False
LAST_RUN = None

_CACHE = {}
_GEOM = {"C": 200}           # columns per chunk; set from the runtime mask


def _build(reps=1):
    C = _GEOM["C"]
    CB = MPJ * C             # SBUF columns per DMA block
    GRID = G * C
    nc = bacc.Bacc(trn_type="TRN2", enable_partition_id=False,
                   num_devices=NCORES)
    xp_d = nc.dram_tensor("xp", [BPC, NJ, 128, CB], BF16,
                          kind="ExternalInput").ap()
    w_d = nc.dram_tensor("wts", [128, NM * 128], BF16,
                         kind="ExternalInput").ap()
    tgt_d = nc.dram_tensor("tgtvec", [128, BPC], F32, kind="ExternalInput").ap()
    mb_d = nc.dram_tensor("mbgrid", [128, C], F32, kind="ExternalInput").ap()
    on_d = nc.dram_tensor("onesr", [1, 128], F32, kind="ExternalInput").ap()
    oc_d = nc.dram_tensor("onescol", [128, 1], F32, kind="ExternalInput").ap()
    dense = nc.dram_tensor("dense", [BPC, GRID], F32,
                           kind="ExternalOutput").ap()

    with tile.TileContext(nc) as tc, ExitStack() as ctx:
        singles = ctx.enter_context(tc.tile_pool(name="singles", bufs=1))
        xpool = ctx.enter_context(tc.tile_pool(name="xpool", bufs=4))
        zpool = ctx.enter_context(tc.tile_pool(name="zpool", bufs=2))
        ppool = ctx.enter_context(tc.tile_pool(name="ppool", bufs=2))
        stat = ctx.enter_context(tc.tile_pool(name="stat", bufs=8))
        psgrid = ctx.enter_context(tc.tile_pool(name="psg", bufs=3,
                                                space="PSUM"))
        pssml = ctx.enter_context(tc.tile_pool(name="pss", bufs=2,
                                               space="PSUM"))

        w_sb = singles.tile([128, NM * 128], BF16)
        # chunked so the first matmuls don't wait on the full 2 MB load
        wq = NM * 128 // 4
        for q in range(4):
            nc.sync.dma_start(out=w_sb[:, q * wq:(q + 1) * wq],
                              in_=w_d[:, q * wq:(q + 1) * wq])
        mb_sb = singles.tile([128, C], F32)
        nc.sync.dma_start(out=mb_sb, in_=mb_d)
        tgt_sb = singles.tile([128, BPC], F32)
        nc.sync.dma_start(out=tgt_sb, in_=tgt_d)
        onesr = singles.tile([1, 128], F32)
        nc.sync.dma_start(out=onesr, in_=on_d)
        onesc = singles.tile([128, 1], F32)
        nc.sync.dma_start(out=onesc, in_=oc_d)

        for _ in range(reps):
            for b in range(BPC):
                xb = xpool.tile([128, NJ * CB], BF16)
                for j in range(NJ):
                    # alternate the two HWDGE rings (SP / ACT)
                    eng = nc.sync if (b * NJ + j) % 2 == 0 else nc.scalar
                    eng.dma_start(out=xb[:, j * CB:(j + 1) * CB],
                                  in_=xp_d[b, j])
                ps = psgrid.tile([128, C], F32)
                for m in range(NM):
                    nc.tensor.matmul(ps,
                                     w_sb[:, m * 128:(m + 1) * 128],
                                     xb[:, m * C:(m + 1) * C],
                                     start=(m == 0), stop=(m == NM - 1))
                # z = scores + tgt (ScalarE copy w/ per-partition bias)
                z = zpool.tile([128, C], F32)
                nc.scalar.activation(z, ps, ACT.Identity,
                                     bias=tgt_sb[:, b:b + 1], scale=1.0)
                # leaky_relu + additive pad mask (-9e15 at pad slots)
                nc.vector.scalar_tensor_tensor(z, z, 0.01, z,
                                               op0=OP.mult, op1=OP.max)
                nc.vector.tensor_add(z, z, mb_sb)
                # exp + per-partition row sums; scores are O(10) so no
                # max-subtraction is needed (softmax is shift-invariant)
                pb = ppool.tile([128, C], F32)
                srow = stat.tile([128, 1], F32)
                nc.scalar.activation(pb, z, ACT.Exp, bias=0.0, scale=1.0,
                                     accum_out=srow)
                # global sum via ones-matmul, reciprocal, broadcast back
                gs = pssml.tile([1, 1], F32, tag="gs")
                nc.tensor.matmul(gs, srow, onesc, start=True, stop=True)
                rec1 = stat.tile([1, 1], F32, tag="rec1")
                nc.vector.reciprocal(rec1, gs)
                bc = pssml.tile([128, 1], F32, tag="bc")
                nc.tensor.matmul(bc, onesr, rec1, start=True, stop=True)
                rec = stat.tile([128, 1], F32, tag="rec")
                nc.vector.tensor_copy(rec, bc)
                nc.vector.tensor_scalar_mul(pb, pb, rec)

                nc.scalar.dma_start(
                    out=dense[b].rearrange("(p t) -> p t", t=C),
                    in_=pb)
    nc.compile()
    return nc


def _geom_from_mask(adj):
    nk = int(np.count_nonzero(adj > 0))
    c = max(8, -(-nk // G))
    c = -(-c // 4) * 4          # multiple of 4 keeps CB/DMA blocks tidy
    assert c <= 512, "psum bank limit"
    return c, nk


def _host_prep(x, a, node_index, adj_mask):
    x = np.asarray(x, dtype=np.float32)
    a = np.asarray(a, dtype=np.float32).reshape(2 * D)
    adj = np.asarray(adj_mask)
    idx = int(node_index)
    a_tgt, a_src = a[:D], a[D:]

    C, nk = _geom_from_mask(adj)
    _GEOM["C"] = C
    GRID = G * C
    kept = np.nonzero(adj > 0)[0]

    tgt = (x[:, idx, :] @ a_tgt).astype(np.float32)          # [B]
    ab = a_src.astype(NPBF16)

    # compact to kept nodes, bf16, pad to GRID, pack into SBUF tile images:
    # xp[b, j, n2*64+d, mw*C+t] = xk[b, (2*(MPJ*j+mw)+n2)*C + t, d]
    xq = np.zeros((B, GRID, D), dtype=NPBF16)
    xq[:, :nk] = x[:, kept, :]
    xp = np.ascontiguousarray(
        xq.reshape(B, NJ, MPJ, 2, C, D).transpose(0, 1, 3, 5, 2, 4)
    ).reshape(B, NJ, 128, MPJ * C)

    # block-sparse weights: W[n2*64+d, m*128+p] = a_src[d] iff p == 2m+n2
    W3 = np.zeros((2, D, NM, 128), dtype=NPBF16)
    mi = np.arange(NM)
    W3[0, :, mi, 2 * mi] = ab[None, :]
    W3[1, :, mi, 2 * mi + 1] = ab[None, :]
    W = W3.reshape(128, NM * 128)

    # additive mask grid in the score layout: NEG at pad slots (>= nk)
    slot = np.arange(GRID).reshape(G, C)
    mb = np.where(slot < nk, 0.0, NEG).astype(np.float32)

    onesr = np.ones((1, 128), dtype=np.float32)
    onesc = np.ones((128, 1), dtype=np.float32)
    return xp, tgt, W, mb, onesr, onesc


def _in_maps(xp, tgt, W, mb, onesr, onesc):
    maps = []
    for c in range(NCORES):
        tv = np.tile(tgt[c * BPC:(c + 1) * BPC][None, :],
                     (128, 1)).astype(np.float32)
        maps.append({
            "xp": xp[c * BPC:(c + 1) * BPC],
            "wts": W,
            "tgtvec": tv,
            "mbgrid": mb,
            "onesr": onesr,
            "onescol": onesc,
        })
    return maps


def _runner():
    """Build the Bass program once per geometry and wrap its NEFF custom
    call in a jitted shard_map over the 8 cores."""
    key = ("runner", _GEOM["C"])
    if key in _CACHE:
        return _CACHE[key]
    install_neuronx_cc_hook()
    nc = _build()
    in_names, out_names, out_avals, zero_shapes = [], [], [], []
    for alloc in nc.m.functions[0].allocations:
        if not isinstance(alloc, mybir.MemoryLocationSet):
            continue
        name = alloc.memorylocations[0].name
        if alloc.kind == "ExternalInput":
            in_names.append(name)
        elif alloc.kind == "ExternalOutput":
            out_names.append(name)
            shape = tuple(alloc.tensor_shape)
            dtype = mybir.dt.np(alloc.dtype)
            out_avals.append(jax.core.ShapedArray(shape, dtype))
            zero_shapes.append((shape, dtype))

    def _body(*args):
        return tuple(_bass_exec_p.bind(
            *args,
            out_avals=tuple(out_avals),
            in_names=tuple(in_names + out_names),
            out_names=tuple(out_names),
            lowering_input_output_aliases=(),
            sim_require_finite=True,
            sim_require_nnan=True,
            nc=nc,
        ))

    mesh = Mesh(np.asarray(jax.devices()[:NCORES]), ("core",))
    nin = len(in_names) + len(out_names)
    sharded = jax.jit(shard_map(
        _body, mesh=mesh,
        in_specs=(PartitionSpec("core"),) * nin,
        out_specs=(PartitionSpec("core"),) * len(out_names),
        check_rep=False))
    _CACHE[key] = (sharded, in_names, out_names, zero_shapes)
    return _CACHE[key]


def kernel(x, a, node_index, adj_mask):
    global LAST_RUN
    adj = np.asarray(adj_mask)
    prep = _host_prep(x, a, node_index, adj_mask)
    maps = _in_maps(*prep)
    sharded, in_names, out_names, zero_shapes = _runner()
    # concat of the 8 per-core xp shards is exactly the full packed x
    ins = [prep[0] if nm == "xp" else
           np.concatenate([m[nm] for m in maps], axis=0) for nm in in_names]
    zeros = [np.zeros((NCORES * s[0], *s[1:]), d) for s, d in zero_shapes]
    outs = sharded(*ins, *zeros)
    LAST_RUN = outs
    dense = np.asarray(outs[out_names.index("dense")])  # [NCORES*BPC, GRID]
    dense = dense.reshape(B, -1)
    kept = np.nonzero(adj > 0)[0]
    attn = np.zeros((B, N), dtype=np.float32)
    attn[:, kept] = dense[:, :len(kept)] * adj[kept].astype(np.float32)
    return attn


# revision 6
# speedup vs baseline: 8.5302x; 1.3903x over previous
"""GNN attention (GAT-style single-target-node) kernel for 8 Trainium2 cores.

Problem:  x [32, 50000, 64], a [128, 1], node_index scalar, adj_mask [50000]
  tgt_score = x[:, idx] @ a[:64]                             # [B]
  e = leaky_relu(tgt_score[:, None] + x @ a[64:], 0.01)      # [B, N]
  attention = softmax(where(adj>0, e, -9e15), axis=1) * adj  # [B, N]

Sharding: data-parallel over batch (32 = 8 cores x 4 batches/core).

Masked nodes (adj_mask == 0) contribute exactly nothing: their attention
is 0 and they drop out of the softmax sum (exp(-9e15) == 0 in fp32).  The
host therefore compacts x to the kept nodes only (halving HBM traffic for
a ~50% mask), and scatters the dense device output back into the full
[B, N] grid.  Geometry is specialized to the runtime kept-count and the
compiled program is cached per size.

The x @ a_src dot product runs on the TensorEngine: the host packs the
kept nodes (cast to bf16, zero-padded to 128*C) into per-batch SBUF tile
images laid out [128 partitions = (node-parity, feature), columns] so
that 64 accumulating matmuls against a block-sparse replicated-a weight
matrix land the scores directly as a [128, C] PSUM grid (psum[p, t] =
score of kept node p*C + t).  Each matmul's stationary operand has a_src
in two one-hot columns, so matmul m adds chunk-2m/2m+1 scores into PSUM
partitions 2m/2m+1 and zeros elsewhere.  ScalarE applies the target
bias + exp (with per-partition row sums), DVE does leaky-relu/pad-mask/
final scale, and PE matmuls handle the cross-partition softmax sum and
broadcast.  Input DMAs alternate between the SP and ACT HWDGE rings.
bf16 x/a gives ~4e-3 rel err vs the 2e-2 gate (fp32 accumulation).
"""

import math
import numpy as np
from contextlib import ExitStack

import ml_dtypes
import jax
from jax.sharding import Mesh, PartitionSpec
from jax.experimental.shard_map import shard_map

import concourse.bass as bass
import concourse.bacc as bacc
import concourse.tile as tile
from concourse import mybir
from concourse.bass2jax import _bass_exec_p, install_neuronx_cc_hook

B, N, D = 32, 50000, 64
NCORES = 8
BPC = B // NCORES            # batches per core
G = 128                      # node chunks per batch = psum partitions
NM = 64                      # matmuls per batch, 2 chunks each
NJ = 2                       # input DMA blocks per batch (one per HWDGE ring)
MPJ = NM // NJ               # matmuls covered per DMA block
NEG = -9.0e15

F32 = mybir.dt.float32
BF16 = mybir.dt.bfloat16
NPBF16 = ml_dtypes.bfloat16
OP = mybir.AluOpType
ACT = mybir.ActivationFunctionType

TRACE = False
LAST_RUN = None

_CACHE = {}
_GEOM = {"C": 200}           # columns per chunk; set from the runtime mask


def _build(reps=1):
    C = _GEOM["C"]
    CB = MPJ * C             # SBUF columns per DMA block
    GRID = G * C
    nc = bacc.Bacc(trn_type="TRN2", enable_partition_id=False,
                   num_devices=NCORES)
    xp_d = nc.dram_tensor("xp", [BPC, NJ, 128, CB], BF16,
                          kind="ExternalInput").ap()
    w_d = nc.dram_tensor("wts", [128, NM * 128], BF16,
                         kind="ExternalInput").ap()
    tgt_d = nc.dram_tensor("tgtvec", [128, BPC], F32, kind="ExternalInput").ap()
    mb_d = nc.dram_tensor("mbgrid", [128, C], F32, kind="ExternalInput").ap()
    on_d = nc.dram_tensor("onesr", [1, 128], F32, kind="ExternalInput").ap()
    oc_d = nc.dram_tensor("onescol", [128, 1], F32, kind="ExternalInput").ap()
    dense = nc.dram_tensor("dense", [BPC, GRID], F32,
                           kind="ExternalOutput").ap()

    with tile.TileContext(nc) as tc, ExitStack() as ctx:
        singles = ctx.enter_context(tc.tile_pool(name="singles", bufs=1))
        xpool = ctx.enter_context(tc.tile_pool(name="xpool", bufs=4))
        zpool = ctx.enter_context(tc.tile_pool(name="zpool", bufs=2))
        ppool = ctx.enter_context(tc.tile_pool(name="ppool", bufs=2))
        stat = ctx.enter_context(tc.tile_pool(name="stat", bufs=8))
        psgrid = ctx.enter_context(tc.tile_pool(name="psg", bufs=3,
                                                space="PSUM"))
        pssml = ctx.enter_context(tc.tile_pool(name="pss", bufs=2,
                                               space="PSUM"))

        w_sb = singles.tile([128, NM * 128], BF16)
        # chunked so the first matmuls don't wait on the full 2 MB load
        wq = NM * 128 // 4
        for q in range(4):
            nc.sync.dma_start(out=w_sb[:, q * wq:(q + 1) * wq],
                              in_=w_d[:, q * wq:(q + 1) * wq])
        mb_sb = singles.tile([128, C], F32)
        nc.sync.dma_start(out=mb_sb, in_=mb_d)
        tgt_sb = singles.tile([128, BPC], F32)
        nc.sync.dma_start(out=tgt_sb, in_=tgt_d)
        onesr = singles.tile([1, 128], F32)
        nc.sync.dma_start(out=onesr, in_=on_d)
        onesc = singles.tile([128, 1], F32)
        nc.sync.dma_start(out=onesc, in_=oc_d)

        for _ in range(reps):
            for b in range(BPC):
                xb = xpool.tile([128, NJ * CB], BF16)
                for j in range(NJ):
                    # alternate the two HWDGE rings (SP / ACT)
                    eng = nc.sync if (b * NJ + j) % 2 == 0 else nc.scalar
                    eng.dma_start(out=xb[:, j * CB:(j + 1) * CB],
                                  in_=xp_d[b, j])
                ps = psgrid.tile([128, C], F32)
                for m in range(NM):
                    nc.tensor.matmul(ps,
                                     w_sb[:, m * 128:(m + 1) * 128],
                                     xb[:, m * C:(m + 1) * C],
                                     start=(m == 0), stop=(m == NM - 1))
                # z = scores + tgt (ScalarE copy w/ per-partition bias)
                z = zpool.tile([128, C], F32)
                nc.scalar.activation(z, ps, ACT.Identity,
                                     bias=tgt_sb[:, b:b + 1], scale=1.0)
                # leaky_relu + additive pad mask (-9e15 at pad slots)
                nc.vector.scalar_tensor_tensor(z, z, 0.01, z,
                                               op0=OP.mult, op1=OP.max)
                nc.vector.tensor_add(z, z, mb_sb)
                # exp + per-partition row sums; scores are O(10) so no
                # max-subtraction is needed (softmax is shift-invariant)
                pb = ppool.tile([128, C], F32)
                srow = stat.tile([128, 1], F32)
                nc.scalar.activation(pb, z, ACT.Exp, bias=0.0, scale=1.0,
                                     accum_out=srow)
                # global sum via ones-matmul, reciprocal, broadcast back
                gs = pssml.tile([1, 1], F32, tag="gs")
                nc.tensor.matmul(gs, srow, onesc, start=True, stop=True)
                rec1 = stat.tile([1, 1], F32, tag="rec1")
                nc.vector.reciprocal(rec1, gs)
                bc = pssml.tile([128, 1], F32, tag="bc")
                nc.tensor.matmul(bc, onesr, rec1, start=True, stop=True)
                rec = stat.tile([128, 1], F32, tag="rec")
                nc.vector.tensor_copy(rec, bc)
                nc.vector.tensor_scalar_mul(pb, pb, rec)

                nc.scalar.dma_start(
                    out=dense[b].rearrange("(p t) -> p t", t=C),
                    in_=pb)
    nc.compile()
    return nc


def _geom_from_mask(adj):
    nk = int(np.count_nonzero(adj > 0))
    c = max(8, -(-nk // G))
    c = -(-c // 4) * 4          # multiple of 4 keeps CB/DMA blocks tidy
    assert c <= 512, "psum bank limit"
    return c, nk


def _host_prep(x, a, node_index, adj_mask):
    x = np.asarray(x, dtype=np.float32)
    a = np.asarray(a, dtype=np.float32).reshape(2 * D)
    adj = np.asarray(adj_mask)
    idx = int(node_index)
    a_tgt, a_src = a[:D], a[D:]

    C, nk = _geom_from_mask(adj)
    _GEOM["C"] = C
    GRID = G * C
    kept = np.nonzero(adj > 0)[0]

    tgt = (x[:, idx, :] @ a_tgt).astype(np.float32)          # [B]
    ab = a_src.astype(NPBF16)

    # compact to kept nodes, bf16, pad to GRID, pack into SBUF tile images:
    # xp[b, j, n2*64+d, mw*C+t] = xk[b, (2*(MPJ*j+mw)+n2)*C + t, d]
    xq = np.zeros((B, GRID, D), dtype=NPBF16)
    xq[:, :nk] = x[:, kept, :]
    xp = np.ascontiguousarray(
        xq.reshape(B, NJ, MPJ, 2, C, D).transpose(0, 1, 3, 5, 2, 4)
    ).reshape(B, NJ, 128, MPJ * C)

    # block-sparse weights: W[n2*64+d, m*128+p] = a_src[d] iff p == 2m+n2
    W3 = np.zeros((2, D, NM, 128), dtype=NPBF16)
    mi = np.arange(NM)
    W3[0, :, mi, 2 * mi] = ab[None, :]
    W3[1, :, mi, 2 * mi + 1] = ab[None, :]
    W = W3.reshape(128, NM * 128)

    # additive mask grid in the score layout: NEG at pad slots (>= nk)
    slot = np.arange(GRID).reshape(G, C)
    mb = np.where(slot < nk, 0.0, NEG).astype(np.float32)

    onesr = np.ones((1, 128), dtype=np.float32)
    onesc = np.ones((128, 1), dtype=np.float32)
    return xp, tgt, W, mb, onesr, onesc


def _in_maps(xp, tgt, W, mb, onesr, onesc):
    maps = []
    for c in range(NCORES):
        tv = np.tile(tgt[c * BPC:(c + 1) * BPC][None, :],
                     (128, 1)).astype(np.float32)
        maps.append({
            "xp": xp[c * BPC:(c + 1) * BPC],
            "wts": W,
            "tgtvec": tv,
            "mbgrid": mb,
            "onesr": onesr,
            "onescol": onesc,
        })
    return maps


def _runner():
    """Build the Bass program once per geometry and wrap its NEFF custom
    call in a jitted shard_map over the 8 cores."""
    key = ("runner", _GEOM["C"])
    if key in _CACHE:
        return _CACHE[key]
    install_neuronx_cc_hook()
    nc = _build()
    in_names, out_names, out_avals, zero_shapes = [], [], [], []
    for alloc in nc.m.functions[0].allocations:
        if not isinstance(alloc, mybir.MemoryLocationSet):
            continue
        name = alloc.memorylocations[0].name
        if alloc.kind == "ExternalInput":
            in_names.append(name)
        elif alloc.kind == "ExternalOutput":
            out_names.append(name)
            shape = tuple(alloc.tensor_shape)
            dtype = mybir.dt.np(alloc.dtype)
            out_avals.append(jax.core.ShapedArray(shape, dtype))
            zero_shapes.append((shape, dtype))

    def _body(*args):
        return tuple(_bass_exec_p.bind(
            *args,
            out_avals=tuple(out_avals),
            in_names=tuple(in_names + out_names),
            out_names=tuple(out_names),
            lowering_input_output_aliases=(),
            sim_require_finite=True,
            sim_require_nnan=True,
            nc=nc,
        ))

    mesh = Mesh(np.asarray(jax.devices()[:NCORES]), ("core",))
    nin = len(in_names) + len(out_names)
    sharded = jax.jit(shard_map(
        _body, mesh=mesh,
        in_specs=(PartitionSpec("core"),) * nin,
        out_specs=(PartitionSpec("core"),) * len(out_names),
        check_rep=False))
    _CACHE[key] = (sharded, in_names, out_names, zero_shapes)
    return _CACHE[key]


def kernel(x, a, node_index, adj_mask):
    global LAST_RUN
    adj = np.asarray(adj_mask)
    prep = _host_prep(x, a, node_index, adj_mask)
    maps = _in_maps(*prep)
    sharded, in_names, out_names, zero_shapes = _runner()
    # concat of the 8 per-core xp shards is exactly the full packed x
    ins = [prep[0] if nm == "xp" else
           np.concatenate([m[nm] for m in maps], axis=0) for nm in in_names]
    zeros = [np.zeros((NCORES * s[0], *s[1:]), d) for s, d in zero_shapes]
    outs = sharded(*ins, *zeros)
    LAST_RUN = outs
    dense = np.asarray(outs[out_names.index("dense")])  # [NCORES*BPC, GRID]
    dense = dense.reshape(B, -1)
    kept = np.nonzero(adj > 0)[0]
    attn = np.zeros((B, N), dtype=np.float32)
    attn[:, kept] = dense[:, :len(kept)] * adj[kept].astype(np.float32)
    return attn


# revision 13
# speedup vs baseline: 8.6673x; 1.0161x over previous
"""GNN attention (GAT-style single-target-node) kernel for 8 Trainium2 cores.

Problem:  x [32, 50000, 64], a [128, 1], node_index scalar, adj_mask [50000]
  tgt_score = x[:, idx] @ a[:64]                             # [B]
  e = leaky_relu(tgt_score[:, None] + x @ a[64:], 0.01)      # [B, N]
  attention = softmax(where(adj>0, e, -9e15), axis=1) * adj  # [B, N]

Sharding: data-parallel over batch (32 = 8 cores x 4 batches/core).

Masked nodes (adj_mask == 0) contribute exactly nothing: their attention
is 0 and they drop out of the softmax sum (exp(-9e15) == 0 in fp32).  The
host therefore compacts x to the kept nodes only (halving HBM traffic for
a ~50% mask), and scatters the dense device output back into the full
[B, N] grid.  Geometry is specialized to the runtime kept-count and the
compiled program is cached per size.

The x @ a_src dot product runs on the TensorEngine: the host packs the
kept nodes (cast to bf16, zero-padded to 128*C) into per-batch SBUF tile
images laid out [128 partitions = (node-parity, feature), columns] so
that 64 accumulating matmuls against a block-sparse replicated-a weight
matrix land the scores directly as a [128, C] PSUM grid (psum[p, t] =
score of kept node p*C + t).  Each matmul's stationary operand has a_src
in two one-hot columns, so matmul m adds chunk-2m/2m+1 scores into PSUM
partitions 2m/2m+1 and zeros elsewhere.  ScalarE applies the target
bias + exp (with per-partition row sums), DVE does leaky-relu/pad-mask/
final scale, and PE matmuls handle the cross-partition softmax sum and
broadcast.  Input DMAs alternate between the SP and ACT HWDGE rings.
bf16 x/a gives ~4e-3 rel err vs the 2e-2 gate (fp32 accumulation).
"""

import math
import numpy as np
from contextlib import ExitStack

import ml_dtypes
import jax
from jax.sharding import Mesh, PartitionSpec
from jax.experimental.shard_map import shard_map

import concourse.bass as bass
import concourse.bacc as bacc
import concourse.tile as tile
from concourse import mybir
from concourse.bass2jax import _bass_exec_p, install_neuronx_cc_hook

B, N, D = 32, 50000, 64
NCORES = 8
BPC = B // NCORES            # batches per core
G = 128                      # node chunks per batch = psum partitions
NM = 64                      # matmuls per batch, 2 chunks each
NJ = 4                       # input DMA blocks per batch (alternating HWDGE rings)
MPJ = NM // NJ               # matmuls covered per DMA block
NEG = -9.0e15

F32 = mybir.dt.float32
BF16 = mybir.dt.bfloat16
NPBF16 = ml_dtypes.bfloat16
OP = mybir.AluOpType
ACT = mybir.ActivationFunctionType

TRACE = False
LAST_RUN = None

_CACHE = {}
_GEOM = {"C": 200}           # columns per chunk; set from the runtime mask


def _build(reps=1):
    C = _GEOM["C"]
    CB = MPJ * C             # SBUF columns per DMA block
    GRID = G * C
    nc = bacc.Bacc(trn_type="TRN2", enable_partition_id=False,
                   num_devices=NCORES)
    xp_d = nc.dram_tensor("xp", [BPC, NJ, 128, CB], BF16,
                          kind="ExternalInput").ap()
    w_d = nc.dram_tensor("wts", [128, NM * 128], BF16,
                         kind="ExternalInput").ap()
    tgt_d = nc.dram_tensor("tgtvec", [128, BPC], F32, kind="ExternalInput").ap()
    mb_d = nc.dram_tensor("mbgrid", [128, C], F32, kind="ExternalInput").ap()
    on_d = nc.dram_tensor("onesr", [1, 128], F32, kind="ExternalInput").ap()
    oc_d = nc.dram_tensor("onescol", [128, 1], F32, kind="ExternalInput").ap()
    dense = nc.dram_tensor("dense", [BPC, GRID], BF16,
                           kind="ExternalOutput").ap()

    with tile.TileContext(nc) as tc, ExitStack() as ctx:
        singles = ctx.enter_context(tc.tile_pool(name="singles", bufs=1))
        xpool = ctx.enter_context(tc.tile_pool(name="xpool", bufs=3 * NJ))
        zpool = ctx.enter_context(tc.tile_pool(name="zpool", bufs=2))
        ppool = ctx.enter_context(tc.tile_pool(name="ppool", bufs=2))
        stat = ctx.enter_context(tc.tile_pool(name="stat", bufs=8))
        psgrid = ctx.enter_context(tc.tile_pool(name="psg", bufs=3,
                                                space="PSUM"))
        pssml = ctx.enter_context(tc.tile_pool(name="pss", bufs=2,
                                               space="PSUM"))

        w_sb = singles.tile([128, NM * 128], BF16)
        # chunked so the first matmuls don't wait on the full 2 MB load
        wq = NM * 128 // 4
        for q in range(4):
            nc.sync.dma_start(out=w_sb[:, q * wq:(q + 1) * wq],
                              in_=w_d[:, q * wq:(q + 1) * wq])
        mb_sb = singles.tile([128, C], F32)
        nc.sync.dma_start(out=mb_sb, in_=mb_d)
        tgt_sb = singles.tile([128, BPC], F32)
        nc.sync.dma_start(out=tgt_sb, in_=tgt_d)
        onesr = singles.tile([1, 128], F32)
        nc.sync.dma_start(out=onesr, in_=on_d)
        onesc = singles.tile([128, 1], F32)
        nc.sync.dma_start(out=onesc, in_=oc_d)

        for _ in range(reps):
            for b in range(BPC):
                # one x tile per DMA block: matmuls for block j only wait on
                # DMA j, so PE work arrives continuously (keeps HAM warm)
                xbs = []
                for j in range(NJ):
                    xb = xpool.tile([128, CB], BF16)
                    # alternate the two HWDGE rings (SP / ACT)
                    eng = nc.sync if (b * NJ + j) % 2 == 0 else nc.scalar
                    eng.dma_start(out=xb, in_=xp_d[b, j])
                    xbs.append(xb)
                ps = psgrid.tile([128, C], F32)
                for m in range(NM):
                    mw = m % MPJ
                    nc.tensor.matmul(ps,
                                     w_sb[:, m * 128:(m + 1) * 128],
                                     xbs[m // MPJ][:, mw * C:(mw + 1) * C],
                                     start=(m == 0), stop=(m == NM - 1))
                # z = scores + tgt (ScalarE copy w/ per-partition bias)
                z = zpool.tile([128, C], F32)
                nc.scalar.activation(z, ps, ACT.Identity,
                                     bias=tgt_sb[:, b:b + 1], scale=1.0)
                # leaky_relu + additive pad mask (-9e15 at pad slots)
                nc.vector.scalar_tensor_tensor(z, z, 0.01, z,
                                               op0=OP.mult, op1=OP.max)
                nc.vector.tensor_add(z, z, mb_sb)
                # exp + per-partition row sums; scores are O(10) so no
                # max-subtraction is needed (softmax is shift-invariant)
                pb = ppool.tile([128, C], F32)
                srow = stat.tile([128, 1], F32)
                nc.scalar.activation(pb, z, ACT.Exp, bias=0.0, scale=1.0,
                                     accum_out=srow)
                # global sum via ones-matmul, reciprocal, broadcast back
                gs = pssml.tile([1, 1], F32, tag="gs")
                nc.tensor.matmul(gs, srow, onesc, start=True, stop=True)
                rec1 = stat.tile([1, 1], F32, tag="rec1")
                nc.vector.reciprocal(rec1, gs)
                bc = pssml.tile([128, 1], F32, tag="bc")
                nc.tensor.matmul(bc, onesr, rec1, start=True, stop=True)
                rec = stat.tile([128, 1], F32, tag="rec")
                nc.vector.tensor_copy(rec, bc)
                pbh = ppool.tile([128, C], BF16, tag="pbh")
                nc.vector.tensor_scalar_mul(pbh, pb, rec)

                nc.scalar.dma_start(
                    out=dense[b].rearrange("(p t) -> p t", t=C),
                    in_=pbh)
    nc.compile()
    return nc


def _geom_from_mask(adj):
    nk = int(np.count_nonzero(adj > 0))
    c = max(8, -(-nk // G))
    assert c <= 512, "psum bank limit"
    return c, nk


def _host_prep(x, a, node_index, adj_mask):
    x = np.asarray(x, dtype=np.float32)
    a = np.asarray(a, dtype=np.float32).reshape(2 * D)
    adj = np.asarray(adj_mask)
    idx = int(node_index)
    a_tgt, a_src = a[:D], a[D:]

    C, nk = _geom_from_mask(adj)
    _GEOM["C"] = C
    GRID = G * C
    kept = np.nonzero(adj > 0)[0]

    tgt = (x[:, idx, :] @ a_tgt).astype(np.float32)          # [B]
    ab = a_src.astype(NPBF16)

    # compact to kept nodes, bf16, pad to GRID, pack into SBUF tile images:
    # xp[b, j, n2*64+d, mw*C+t] = xk[b, (2*(MPJ*j+mw)+n2)*C + t, d]
    xq = np.zeros((B, GRID, D), dtype=NPBF16)
    xq[:, :nk] = x[:, kept, :]
    xp = np.ascontiguousarray(
        xq.reshape(B, NJ, MPJ, 2, C, D).transpose(0, 1, 3, 5, 2, 4)
    ).reshape(B, NJ, 128, MPJ * C)

    # block-sparse weights: W[n2*64+d, m*128+p] = a_src[d] iff p == 2m+n2
    W3 = np.zeros((2, D, NM, 128), dtype=NPBF16)
    mi = np.arange(NM)
    W3[0, :, mi, 2 * mi] = ab[None, :]
    W3[1, :, mi, 2 * mi + 1] = ab[None, :]
    W = W3.reshape(128, NM * 128)

    # additive mask grid in the score layout: NEG at pad slots (>= nk)
    slot = np.arange(GRID).reshape(G, C)
    mb = np.where(slot < nk, 0.0, NEG).astype(np.float32)

    onesr = np.ones((1, 128), dtype=np.float32)
    onesc = np.ones((128, 1), dtype=np.float32)
    return xp, tgt, W, mb, onesr, onesc


def _in_maps(xp, tgt, W, mb, onesr, onesc):
    maps = []
    for c in range(NCORES):
        tv = np.tile(tgt[c * BPC:(c + 1) * BPC][None, :],
                     (128, 1)).astype(np.float32)
        maps.append({
            "xp": xp[c * BPC:(c + 1) * BPC],
            "wts": W,
            "tgtvec": tv,
            "mbgrid": mb,
            "onesr": onesr,
            "onescol": onesc,
        })
    return maps


def _runner():
    """Build the Bass program once per geometry and wrap its NEFF custom
    call in a jitted shard_map over the 8 cores."""
    key = ("runner", _GEOM["C"])
    if key in _CACHE:
        return _CACHE[key]
    install_neuronx_cc_hook()
    nc = _build()
    in_names, out_names, out_avals, zero_shapes = [], [], [], []
    for alloc in nc.m.functions[0].allocations:
        if not isinstance(alloc, mybir.MemoryLocationSet):
            continue
        name = alloc.memorylocations[0].name
        if alloc.kind == "ExternalInput":
            in_names.append(name)
        elif alloc.kind == "ExternalOutput":
            out_names.append(name)
            shape = tuple(alloc.tensor_shape)
            dtype = mybir.dt.np(alloc.dtype)
            out_avals.append(jax.core.ShapedArray(shape, dtype))
            zero_shapes.append((shape, dtype))

    def _body(*args):
        return tuple(_bass_exec_p.bind(
            *args,
            out_avals=tuple(out_avals),
            in_names=tuple(in_names + out_names),
            out_names=tuple(out_names),
            lowering_input_output_aliases=(),
            sim_require_finite=True,
            sim_require_nnan=True,
            nc=nc,
        ))

    mesh = Mesh(np.asarray(jax.devices()[:NCORES]), ("core",))
    nin = len(in_names) + len(out_names)
    sharded = jax.jit(shard_map(
        _body, mesh=mesh,
        in_specs=(PartitionSpec("core"),) * nin,
        out_specs=(PartitionSpec("core"),) * len(out_names),
        check_rep=False))
    _CACHE[key] = (sharded, in_names, out_names, zero_shapes)
    return _CACHE[key]


def kernel(x, a, node_index, adj_mask):
    global LAST_RUN
    adj = np.asarray(adj_mask)
    prep = _host_prep(x, a, node_index, adj_mask)
    maps = _in_maps(*prep)
    sharded, in_names, out_names, zero_shapes = _runner()
    # concat of the 8 per-core xp shards is exactly the full packed x
    ins = [prep[0] if nm == "xp" else
           np.concatenate([m[nm] for m in maps], axis=0) for nm in in_names]
    zeros = [np.zeros((NCORES * s[0], *s[1:]), d) for s, d in zero_shapes]
    outs = sharded(*ins, *zeros)
    LAST_RUN = outs
    dense = np.asarray(outs[out_names.index("dense")])  # [NCORES*BPC, GRID]
    dense = dense.reshape(B, -1).astype(np.float32)
    kept = np.nonzero(adj > 0)[0]
    attn = np.zeros((B, N), dtype=np.float32)
    attn[:, kept] = dense[:, :len(kept)] * adj[kept].astype(np.float32)
    return attn


# revision 14
# speedup vs baseline: 10.9357x; 1.2617x over previous
"""GNN attention (GAT-style single-target-node) kernel for 8 Trainium2 cores.

Problem:  x [32, 50000, 64], a [128, 1], node_index scalar, adj_mask [50000]
  tgt_score = x[:, idx] @ a[:64]                             # [B]
  e = leaky_relu(tgt_score[:, None] + x @ a[64:], 0.01)      # [B, N]
  attention = softmax(where(adj>0, e, -9e15), axis=1) * adj  # [B, N]

Sharding: data-parallel over batch (32 = 8 cores x 4 batches/core).

Masked nodes (adj_mask == 0) contribute exactly nothing: their attention
is 0 and they drop out of the softmax sum (exp(-9e15) == 0 in fp32).  The
host therefore compacts x to the kept nodes only (halving HBM traffic for
a ~50% mask), and scatters the dense device output back into the full
[B, N] grid.  Geometry is specialized to the runtime kept-count and the
compiled program is cached per size.

The x @ a_src dot product runs on the TensorEngine: the host packs the
kept nodes (cast to bf16, zero-padded to 128*C) into per-batch SBUF tile
images laid out [128 partitions = (node-parity, feature), columns] so
that 64 accumulating matmuls against a block-sparse replicated-a weight
matrix land the scores directly as a [128, C] PSUM grid (psum[p, t] =
score of kept node p*C + t).  Each matmul's stationary operand has a_src
in two one-hot columns, so matmul m adds chunk-2m/2m+1 scores into PSUM
partitions 2m/2m+1 and zeros elsewhere.  ScalarE applies the target
bias + exp (with per-partition row sums), DVE does leaky-relu/pad-mask/
final scale, and PE matmuls handle the cross-partition softmax sum and
broadcast.  Input DMAs alternate between the SP and ACT HWDGE rings.
bf16 x/a gives ~4e-3 rel err vs the 2e-2 gate (fp32 accumulation).
"""

import math
import numpy as np
from contextlib import ExitStack

import ml_dtypes
import jax
from jax.sharding import Mesh, PartitionSpec
from jax.experimental.shard_map import shard_map

import concourse.bass as bass
import concourse.bacc as bacc
import concourse.tile as tile
from concourse import mybir
from concourse.bass2jax import _bass_exec_p, install_neuronx_cc_hook

B, N, D = 32, 50000, 64
NCORES = 8
BPC = B // NCORES            # batches per core
G = 128                      # node chunks per batch = psum partitions
NM = 64                      # matmuls per batch, 2 chunks each
NJ = 1                       # input DMA blocks per batch (alternating HWDGE rings)
MPJ = NM // NJ               # matmuls covered per DMA block
NEG = -9.0e15

F32 = mybir.dt.float32
BF16 = mybir.dt.bfloat16
NPBF16 = ml_dtypes.bfloat16
OP = mybir.AluOpType
ACT = mybir.ActivationFunctionType

TRACE = False
LAST_RUN = None

_CACHE = {}
_GEOM = {"C": 200}           # columns per chunk; set from the runtime mask


def _build(reps=1):
    C = _GEOM["C"]
    CB = MPJ * C             # SBUF columns per DMA block
    GRID = G * C
    nc = bacc.Bacc(trn_type="TRN2", enable_partition_id=False,
                   num_devices=NCORES)
    xp_d = nc.dram_tensor("xp", [BPC, NJ, 128, CB], BF16,
                          kind="ExternalInput").ap()
    w_d = nc.dram_tensor("wts", [128, NM * 128], BF16,
                         kind="ExternalInput").ap()
    tgt_d = nc.dram_tensor("tgtvec", [128, BPC], F32, kind="ExternalInput").ap()
    mb_d = nc.dram_tensor("mbgrid", [128, C], F32, kind="ExternalInput").ap()
    on_d = nc.dram_tensor("onesr", [1, 128], F32, kind="ExternalInput").ap()
    oc_d = nc.dram_tensor("onescol", [128, 1], F32, kind="ExternalInput").ap()
    dense = nc.dram_tensor("dense", [BPC, GRID], BF16,
                           kind="ExternalOutput").ap()

    with tile.TileContext(nc) as tc, ExitStack() as ctx:
        singles = ctx.enter_context(tc.tile_pool(name="singles", bufs=1))
        xpool = ctx.enter_context(tc.tile_pool(name="xpool", bufs=3 * NJ))
        zpool = ctx.enter_context(tc.tile_pool(name="zpool", bufs=2))
        ppool = ctx.enter_context(tc.tile_pool(name="ppool", bufs=2))
        stat = ctx.enter_context(tc.tile_pool(name="stat", bufs=8))
        psgrid = ctx.enter_context(tc.tile_pool(name="psg", bufs=3,
                                                space="PSUM"))
        pssml = ctx.enter_context(tc.tile_pool(name="pss", bufs=2,
                                               space="PSUM"))

        w_sb = singles.tile([128, NM * 128], BF16)
        # chunked so the first matmuls don't wait on the full 2 MB load
        wq = NM * 128 // 4
        for q in range(4):
            nc.sync.dma_start(out=w_sb[:, q * wq:(q + 1) * wq],
                              in_=w_d[:, q * wq:(q + 1) * wq])
        mb_sb = singles.tile([128, C], F32)
        nc.sync.dma_start(out=mb_sb, in_=mb_d)
        tgt_sb = singles.tile([128, BPC], F32)
        nc.sync.dma_start(out=tgt_sb, in_=tgt_d)
        onesr = singles.tile([1, 128], F32)
        nc.sync.dma_start(out=onesr, in_=on_d)
        onesc = singles.tile([128, 1], F32)
        nc.sync.dma_start(out=onesc, in_=oc_d)

        for _ in range(reps):
            for b in range(BPC):
                # one x tile per DMA block: matmuls for block j only wait on
                # DMA j, so PE work arrives continuously (keeps HAM warm)
                xbs = []
                for j in range(NJ):
                    xb = xpool.tile([128, CB], BF16)
                    # alternate the two HWDGE rings (SP / ACT)
                    eng = nc.sync if (b * NJ + j) % 2 == 0 else nc.scalar
                    eng.dma_start(out=xb, in_=xp_d[b, j])
                    xbs.append(xb)
                ps = psgrid.tile([128, C], F32)
                for m in range(NM):
                    mw = m % MPJ
                    nc.tensor.matmul(ps,
                                     w_sb[:, m * 128:(m + 1) * 128],
                                     xbs[m // MPJ][:, mw * C:(mw + 1) * C],
                                     start=(m == 0), stop=(m == NM - 1))
                # z = scores + tgt (ScalarE copy w/ per-partition bias)
                z = zpool.tile([128, C], F32)
                nc.scalar.activation(z, ps, ACT.Identity,
                                     bias=tgt_sb[:, b:b + 1], scale=1.0)
                # leaky_relu + additive pad mask (-9e15 at pad slots)
                nc.vector.scalar_tensor_tensor(z, z, 0.01, z,
                                               op0=OP.mult, op1=OP.max)
                nc.vector.tensor_add(z, z, mb_sb)
                # exp + per-partition row sums; scores are O(10) so no
                # max-subtraction is needed (softmax is shift-invariant)
                pb = ppool.tile([128, C], F32)
                srow = stat.tile([128, 1], F32)
                nc.scalar.activation(pb, z, ACT.Exp, bias=0.0, scale=1.0,
                                     accum_out=srow)
                # global sum via ones-matmul, reciprocal, broadcast back
                gs = pssml.tile([1, 1], F32, tag="gs")
                nc.tensor.matmul(gs, srow, onesc, start=True, stop=True)
                rec1 = stat.tile([1, 1], F32, tag="rec1")
                nc.vector.reciprocal(rec1, gs)
                bc = pssml.tile([128, 1], F32, tag="bc")
                nc.tensor.matmul(bc, onesr, rec1, start=True, stop=True)
                rec = stat.tile([128, 1], F32, tag="rec")
                nc.vector.tensor_copy(rec, bc)
                pbh = ppool.tile([128, C], BF16, tag="pbh")
                nc.vector.tensor_scalar_mul(pbh, pb, rec)

                nc.scalar.dma_start(
                    out=dense[b].rearrange("(p t) -> p t", t=C),
                    in_=pbh)
    nc.compile()
    return nc


def _geom_from_mask(adj):
    nk = int(np.count_nonzero(adj > 0))
    c = max(8, -(-nk // G))
    assert c <= 512, "psum bank limit"
    return c, nk


def _host_prep(x, a, node_index, adj_mask):
    x = np.asarray(x, dtype=np.float32)
    a = np.asarray(a, dtype=np.float32).reshape(2 * D)
    adj = np.asarray(adj_mask)
    idx = int(node_index)
    a_tgt, a_src = a[:D], a[D:]

    C, nk = _geom_from_mask(adj)
    _GEOM["C"] = C
    GRID = G * C
    kept = np.nonzero(adj > 0)[0]

    tgt = (x[:, idx, :] @ a_tgt).astype(np.float32)          # [B]
    ab = a_src.astype(NPBF16)

    # compact to kept nodes, bf16, pad to GRID, pack into SBUF tile images:
    # xp[b, j, n2*64+d, mw*C+t] = xk[b, (2*(MPJ*j+mw)+n2)*C + t, d]
    xq = np.zeros((B, GRID, D), dtype=NPBF16)
    xq[:, :nk] = x[:, kept, :]
    xp = np.ascontiguousarray(
        xq.reshape(B, NJ, MPJ, 2, C, D).transpose(0, 1, 3, 5, 2, 4)
    ).reshape(B, NJ, 128, MPJ * C)

    # block-sparse weights: W[n2*64+d, m*128+p] = a_src[d] iff p == 2m+n2
    W3 = np.zeros((2, D, NM, 128), dtype=NPBF16)
    mi = np.arange(NM)
    W3[0, :, mi, 2 * mi] = ab[None, :]
    W3[1, :, mi, 2 * mi + 1] = ab[None, :]
    W = W3.reshape(128, NM * 128)

    # additive mask grid in the score layout: NEG at pad slots (>= nk)
    slot = np.arange(GRID).reshape(G, C)
    mb = np.where(slot < nk, 0.0, NEG).astype(np.float32)

    onesr = np.ones((1, 128), dtype=np.float32)
    onesc = np.ones((128, 1), dtype=np.float32)
    return xp, tgt, W, mb, onesr, onesc


def _in_maps(xp, tgt, W, mb, onesr, onesc):
    maps = []
    for c in range(NCORES):
        tv = np.tile(tgt[c * BPC:(c + 1) * BPC][None, :],
                     (128, 1)).astype(np.float32)
        maps.append({
            "xp": xp[c * BPC:(c + 1) * BPC],
            "wts": W,
            "tgtvec": tv,
            "mbgrid": mb,
            "onesr": onesr,
            "onescol": onesc,
        })
    return maps


def _runner():
    """Build the Bass program once per geometry and wrap its NEFF custom
    call in a jitted shard_map over the 8 cores."""
    key = ("runner", _GEOM["C"])
    if key in _CACHE:
        return _CACHE[key]
    install_neuronx_cc_hook()
    nc = _build()
    in_names, out_names, out_avals, zero_shapes = [], [], [], []
    for alloc in nc.m.functions[0].allocations:
        if not isinstance(alloc, mybir.MemoryLocationSet):
            continue
        name = alloc.memorylocations[0].name
        if alloc.kind == "ExternalInput":
            in_names.append(name)
        elif alloc.kind == "ExternalOutput":
            out_names.append(name)
            shape = tuple(alloc.tensor_shape)
            dtype = mybir.dt.np(alloc.dtype)
            out_avals.append(jax.core.ShapedArray(shape, dtype))
            zero_shapes.append((shape, dtype))

    def _body(*args):
        return tuple(_bass_exec_p.bind(
            *args,
            out_avals=tuple(out_avals),
            in_names=tuple(in_names + out_names),
            out_names=tuple(out_names),
            lowering_input_output_aliases=(),
            sim_require_finite=True,
            sim_require_nnan=True,
            nc=nc,
        ))

    mesh = Mesh(np.asarray(jax.devices()[:NCORES]), ("core",))
    nin = len(in_names) + len(out_names)
    sharded = jax.jit(shard_map(
        _body, mesh=mesh,
        in_specs=(PartitionSpec("core"),) * nin,
        out_specs=(PartitionSpec("core"),) * len(out_names),
        check_rep=False))
    _CACHE[key] = (sharded, in_names, out_names, zero_shapes)
    return _CACHE[key]


def kernel(x, a, node_index, adj_mask):
    global LAST_RUN
    adj = np.asarray(adj_mask)
    prep = _host_prep(x, a, node_index, adj_mask)
    maps = _in_maps(*prep)
    sharded, in_names, out_names, zero_shapes = _runner()
    # concat of the 8 per-core xp shards is exactly the full packed x
    ins = [prep[0] if nm == "xp" else
           np.concatenate([m[nm] for m in maps], axis=0) for nm in in_names]
    zeros = [np.zeros((NCORES * s[0], *s[1:]), d) for s, d in zero_shapes]
    outs = sharded(*ins, *zeros)
    LAST_RUN = outs
    dense = np.asarray(outs[out_names.index("dense")])  # [NCORES*BPC, GRID]
    dense = dense.reshape(B, -1).astype(np.float32)
    kept = np.nonzero(adj > 0)[0]
    attn = np.zeros((B, N), dtype=np.float32)
    attn[:, kept] = dense[:, :len(kept)] * adj[kept].astype(np.float32)
    return attn
